# revision 32
# baseline (speedup 1.0000x reference)
import os
import sys

import numpy as np

for p in ("/opt/trn_rl_repo",):
    if p not in sys.path:
        sys.path.insert(0, p)

import concourse.bass as bass  # noqa: E402
import concourse.tile as tile  # noqa: E402
from concourse import bacc, mybir  # noqa: E402
from concourse.bass_utils import run_bass_kernel_spmd  # noqa: E402

B, N, D = 128, 512, 512
NCORES = 8
BPC = B // NCORES  # 16 batch items per core
F32 = mybir.dt.float32
F16 = mybir.dt.float16

LAST_RESULTS = None


def _hadamard(n: int) -> np.ndarray:
    H = np.array([[1.0]], dtype=np.float32)
    base = np.array([[1.0, 1.0], [1.0, -1.0]], dtype=np.float32)
    while H.shape[0] < n:
        H = np.kron(H, base)
    return H


def _build():
    nc = bacc.Bacc("TRN2", target_bir_lowering=False, debug=False)
    # x/y as [BPC, 128, 2048] bf16: same bytes as [BPC, 512, 512]; partition
    # p holds rows 4p..4p+3 (column block k of 512 = row 4p+k).
    x_d = nc.dram_tensor("x", [BPC, 128, 4 * D], F16, kind="ExternalInput").ap()
    h128_d = nc.dram_tensor("h128", [128, 128], F16, kind="ExternalInput").ap()
    # h1[p, k*512 + l*128 + q] = H512[4p+k, 4q+l]  (full left H, permuted)
    h1_d = nc.dram_tensor("h1", [128, 4 * N], F16, kind="ExternalInput").ap()
    # h256[c, dtl*256 + s] = H256[dtl*128+c, s] / 512  (right kron factor)
    h256_d = nc.dram_tensor("h256", [128, 512], F16, kind="ExternalInput").ap()
    y_d = nc.dram_tensor("y", [BPC, 128, 4 * D], F16, kind="ExternalOutput").ap()

    with tile.TileContext(nc) as tc:
        with (
            tc.tile_pool(name="const", bufs=1) as const_pool,
            tc.tile_pool(name="xp", bufs=4) as x_pool,
            tc.tile_pool(name="xm", bufs=3) as xm_pool,
            tc.tile_pool(name="xc", bufs=4) as xc_pool,
            tc.tile_pool(name="tp", bufs=5) as t_pool,
            tc.tile_pool(name="yp", bufs=4) as y_pool,
            tc.tile_pool(name="up", bufs=3) as u_pool,
            tc.tile_pool(name="ps", bufs=4, space="PSUM") as psum_pool,
        ):
            W = 4 * D  # 2048, one slice's width
            xts = {}
            xcs = {}
            tts = {}
            yts = {}
            pend = []

            def emit_load(b0, butterfly=True):
                xt = x_pool.tile([128, 2 * W], F16, name="xt")
                if butterfly:
                    nc.sync.dma_start(
                        xt[:].rearrange("p (s j) -> p s j", s=2),
                        x_d[b0 : b0 + 2].transpose([1, 0, 2]),
                    )
                else:
                    # first pair: two parallel 512 KiB DMAs so slice b0 is
                    # ready as early as possible
                    nc.sync.dma_start(xt[:, 0:W], x_d[b0])
                    nc.sync.dma_start(xt[:, W : 2 * W], x_d[b0 + 1])
                yt = y_pool.tile([128, 2 * W], F16, name="yt")
                yts[b0] = yt
                yts[b0 + 1] = yt
                if not butterfly:
                    # first pair goes through the full-H512 left pass; the PE
                    # needs no butterfly results to start
                    xts[b0] = (xt, 0)
                    xts[b0 + 1] = (xt, W)
                    return
                # H4 combine on x: xc_l = sum_k H4[k,l] x_k
                xm = xm_pool.tile([128, 2 * W], F16, name="xm")
                xc = xc_pool.tile([128, 2 * W], F16, name="xc")

                def blk(t, k):
                    return t[:].rearrange("p (s j) -> p s j", s=2)[
                        :, :, k * D : (k + 1) * D
                    ]

                # level 1 all on DVE (POOL never blocks DVE)
                nc.vector.tensor_add(blk(xm, 0), blk(xt, 0), blk(xt, 1))
                nc.vector.tensor_sub(blk(xm, 1), blk(xt, 0), blk(xt, 1))
                nc.vector.tensor_add(blk(xm, 2), blk(xt, 2), blk(xt, 3))
                nc.vector.tensor_sub(blk(xm, 3), blk(xt, 2), blk(xt, 3))
                # level 2 also on DVE — GpSimd shares an SBUF port with DVE
                # (exclusive lock), so any GpSimd op doubles concurrent DVE ops
                nc.vector.tensor_add(blk(xc, 0), blk(xm, 0), blk(xm, 2))
                nc.vector.tensor_add(blk(xc, 1), blk(xm, 1), blk(xm, 3))
                nc.vector.tensor_sub(blk(xc, 2), blk(xm, 0), blk(xm, 2))
                nc.vector.tensor_sub(blk(xc, 3), blk(xm, 1), blk(xm, 3))
                xcs[b0] = (xc, 0)
                xcs[b0 + 1] = (xc, W)

            def emit_stage_a(s):
                # kron path: t_l[c, dt*128+q] = sum_p xc_l[p, dt*128+c] H128[p,q]
                # tt col layout: l*512 + dt*128 + q
                xc, xo = xcs.pop(s)
                tps = [
                    psum_pool.tile([128, 2 * N], F32, name=f"tps{h}", tag="ps")
                    for h in range(2)
                ]
                for l in range(4):
                    for dt_ in range(4):
                        nc.tensor.matmul(
                            tps[l // 2][
                                :, (l % 2) * N + dt_ * 128 : (l % 2) * N + dt_ * 128 + 128
                            ],
                            xc[:, xo + l * D + dt_ * 128 : xo + l * D + dt_ * 128 + 128],
                            h128_sb[:],
                            start=True,
                            stop=True,
                        )
                tt = t_pool.tile([128, 4 * N], F16, name="tt")
                nc.scalar.copy(tt[:, 0 : 2 * N], tps[0][:])
                nc.scalar.copy(tt[:, 2 * N : 4 * N], tps[1][:])
                tts[s] = ("lmaj", tt)

            def emit_stage_a_full(s):
                # full-H512 left pass: tT[d, n'] with col layout dt*512+l*128+q
                xt, xo = xts.pop(s)
                tps = [
                    psum_pool.tile([128, 2 * N], F32, name=f"tps{h}", tag="ps")
                    for h in range(2)
                ]
                for dt_ in range(4):
                    out = tps[dt_ // 2][:, (dt_ % 2) * N : (dt_ % 2 + 1) * N]
                    for k in range(4):
                        nc.tensor.matmul(
                            out,
                            xt[:, xo + k * D + dt_ * 128 : xo + k * D + dt_ * 128 + 128],
                            h1_sb[:, k * N : (k + 1) * N],
                            start=(k == 0),
                            stop=(k == 3),
                        )
                tt = t_pool.tile([128, 4 * N], F16, name="tt")
                nc.scalar.copy(tt[:, 0 : 2 * N], tps[0][:])
                nc.scalar.copy(tt[:, 2 * N : 4 * N], tps[1][:])
                tts[s] = ("dmaj", tt)

            def emit_pass2(s):
                # Right transform via H512 = H2 (x) H256:
                # U_h[p', k2*256+s] = sum_{dt in {2h,2h+1}} sum_c
                #     tT[dt*128+c, 4p'+k2] H256[(dt&1)*128+c, s]/512
                # y[4p'+k2, ehi*256+s] = U_0 + (-1)^ehi U_1
                layout, tt = tts.pop(s)
                yt = yts.pop(s)
                yo = (s % 2) * W
                pps = [
                    psum_pool.tile([128, 2 * D], F32, name=f"pps{h}", tag="ps")
                    for h in range(2)
                ]
                for k2 in range(4):
                    for h in range(2):
                        for dtl in range(2):
                            dt_ = 2 * h + dtl
                            if layout == "lmaj":
                                lhsT = tt[
                                    :, k2 * N + dt_ * 128 : k2 * N + dt_ * 128 + 128
                                ]
                            else:
                                lhsT = tt[
                                    :, dt_ * N + k2 * 128 : dt_ * N + k2 * 128 + 128
                                ]
                            nc.tensor.matmul(
                                pps[h][:, k2 * 256 : (k2 + 1) * 256],
                                lhsT,
                                h256_sb[:, dtl * 256 : (dtl + 1) * 256],
                                start=(dtl == 0),
                                stop=(dtl == 1),
                            )
                u0 = u_pool.tile([128, 2 * D], F16, name="u0")
                u1 = u_pool.tile([128, 2 * D], F16, name="u1")
                nc.scalar.copy(u0[:], pps[0][:])
                if s % 2 == 0:
                    nc.scalar.copy(u1[:], pps[1][:])
                else:
                    nc.vector.tensor_copy(u1[:], pps[1][:])
                ys = yt[:, yo : yo + W].rearrange("p (k e) -> p k e", e=512)
                u0v = u0[:].rearrange("p (k v) -> p k v", v=256)
                u1v = u1[:].rearrange("p (k v) -> p k v", v=256)
                if s >= BPC - 2:
                    # last pair: combine and store in halves so the final
                    # DMAs start as early as possible
                    for hf in range(2):
                        sl = slice(hf * 2, hf * 2 + 2)
                        nc.vector.tensor_add(ys[:, sl, 0:256], u0v[:, sl], u1v[:, sl])
                        nc.vector.tensor_sub(ys[:, sl, 256:512], u0v[:, sl], u1v[:, sl])
                        nc.sync.dma_start(
                            y_d[s][:, hf * 2 * D : (hf * 2 + 2) * D],
                            yt[:, yo + hf * 2 * D : yo + (hf * 2 + 2) * D],
                        )
                    return
                nc.vector.tensor_add(ys[:, :, 0:256], u0v, u1v)
                nc.vector.tensor_sub(ys[:, :, 256:512], u0v, u1v)
                if s % 2 == 1:
                    b0 = s - 1
                    nc.sync.dma_start(
                        y_d[b0 : b0 + 2].transpose([1, 0, 2]),
                        yt[:].rearrange("p (s j) -> p s j", s=2),
                    )

            # x pairs 0-1 and the H constants are prefetched first; slices
            # 0-3 take the full-H512 left pass (no butterfly dependency) so
            # the PE starts early while the butterfly pipeline fills
            emit_load(0, butterfly=False)
            h1_sb = const_pool.tile([128, 4 * N], F16, tag="h1")
            nc.sync.dma_start(h1_sb[:], h1_d[:])
            emit_load(2)  # pair 1 early so its butterflies fill the pipeline
            h256_sb = const_pool.tile([128, 512], F16, tag="h256")
            nc.sync.dma_start(h256_sb[:], h256_d[:])
            h128_sb = const_pool.tile([128, 128], F16, tag="h128")
            nc.sync.dma_start(h128_sb[:], h128_d[:])

            for s in range(BPC):
                if s % 2 == 0 and s > 2:
                    emit_load(s)
                if s < 2:
                    emit_stage_a_full(s)
                else:
                    emit_stage_a(s)
                # pass-2 lags stage-A by 2 slices in steady state; lag 1
                # during startup and drain so the PE never starves
                lag = 1 if (s < 3 or s >= BPC - 2) else 2
                while len(pend) >= lag:
                    emit_pass2(pend.pop(0))
                pend.append(s)
            while pend:
                emit_pass2(pend.pop(0))

    nc.compile()
    return nc


_NC = None


def kernel(x: np.ndarray) -> np.ndarray:
    global _NC, LAST_RESULTS
    if _NC is None:
        _NC = _build()
    x = (
        np.ascontiguousarray(np.asarray(x), dtype=np.float32)
        .astype(np.float16)
        .reshape(NCORES, BPC, 128, 4 * D)
    )
    H = _hadamard(N)
    h128 = np.ascontiguousarray(_hadamard(128)).astype(np.float16)
    h1 = np.ascontiguousarray(
        H.reshape(128, 4, 128, 4).transpose(0, 1, 3, 2).reshape(128, 4 * N)
    ).astype(np.float16)
    H256 = _hadamard(256)
    h256 = np.ascontiguousarray(
        H256.reshape(2, 128, 256).transpose(1, 0, 2).reshape(128, 512)
        / np.float32(512.0)
    ).astype(np.float16)
    in_maps = [{"x": x[i], "h128": h128, "h1": h1, "h256": h256} for i in range(NCORES)]
    trace = os.environ.get("KERNEL_TRACE", "") == "1"
    res = run_bass_kernel_spmd(_NC, in_maps, list(range(NCORES)), trace=trace)
    LAST_RESULTS = res
    out = np.stack([np.asarray(r["y"]) for r in res.results], axis=0)
    return out.reshape(B, N, D).astype(np.float32)


# revision 35
# speedup vs baseline: 1.1717x; 1.1717x over previous
import os
import sys

import numpy as np

for p in ("/opt/trn_rl_repo",):
    if p not in sys.path:
        sys.path.insert(0, p)

import concourse.bass as bass  # noqa: E402
import concourse.tile as tile  # noqa: E402
from concourse import bacc, mybir  # noqa: E402
from concourse.bass_utils import run_bass_kernel_spmd  # noqa: E402

B, N, D = 128, 512, 512
NCORES = 8
BPC = B // NCORES  # 16 batch items per core
F32 = mybir.dt.float32
F16 = mybir.dt.float16

LAST_RESULTS = None


def _hadamard(n: int) -> np.ndarray:
    H = np.array([[1.0]], dtype=np.float32)
    base = np.array([[1.0, 1.0], [1.0, -1.0]], dtype=np.float32)
    while H.shape[0] < n:
        H = np.kron(H, base)
    return H


def _build():
    nc = bacc.Bacc("TRN2", target_bir_lowering=False, debug=False)
    # x/y as [BPC, 128, 2048] bf16: same bytes as [BPC, 512, 512]; partition
    # p holds rows 4p..4p+3 (column block k of 512 = row 4p+k).
    x_d = nc.dram_tensor("x", [BPC, 128, 4 * D], F16, kind="ExternalInput").ap()
    h128_d = nc.dram_tensor("h128", [128, 128], F16, kind="ExternalInput").ap()
    # h1[p, k*512 + l*128 + q] = H512[4p+k, 4q+l]  (full left H, permuted)
    h1_d = nc.dram_tensor("h1", [128, 4 * N], F16, kind="ExternalInput").ap()
    # h256[c, dtl*256 + s] = H256[dtl*128+c, s] / 512  (right kron factor)
    h256_d = nc.dram_tensor("h256", [128, 512], F16, kind="ExternalInput").ap()
    y_d = nc.dram_tensor("y", [BPC, 128, 4 * D], F16, kind="ExternalOutput").ap()

    with tile.TileContext(nc) as tc:
        with (
            tc.tile_pool(name="const", bufs=1) as const_pool,
            tc.tile_pool(name="xp", bufs=4) as x_pool,
            tc.tile_pool(name="xm", bufs=3) as xm_pool,
            tc.tile_pool(name="xc", bufs=4) as xc_pool,
            tc.tile_pool(name="tp", bufs=5) as t_pool,
            tc.tile_pool(name="yp", bufs=4) as y_pool,
            tc.tile_pool(name="up", bufs=3) as u_pool,
            tc.tile_pool(name="ps", bufs=4, space="PSUM") as psum_pool,
        ):
            W = 4 * D  # 2048, one slice's width
            xts = {}
            xcs = {}
            tts = {}
            yts = {}
            pend = []

            def emit_load(b0, butterfly=True):
                xt = x_pool.tile([128, 2 * W], F16, name="xt")
                if butterfly:
                    nc.sync.dma_start(
                        xt[:].rearrange("p (s j) -> p s j", s=2),
                        x_d[b0 : b0 + 2].transpose([1, 0, 2]),
                    )
                else:
                    # first pair: two parallel 512 KiB DMAs so slice b0 is
                    # ready as early as possible
                    nc.sync.dma_start(xt[:, 0:W], x_d[b0])
                    nc.sync.dma_start(xt[:, W : 2 * W], x_d[b0 + 1])
                yt = y_pool.tile([128, 2 * W], F16, name="yt")
                yts[b0] = yt
                yts[b0 + 1] = yt
                if not butterfly:
                    # first pair goes through the full-H512 left pass; the PE
                    # needs no butterfly results to start
                    xts[b0] = (xt, 0)
                    xts[b0 + 1] = (xt, W)
                    return
                # H4 combine on x: xc_l = sum_k H4[k,l] x_k
                xm = xm_pool.tile([128, 2 * W], F16, name="xm")
                xc = xc_pool.tile([128, 2 * W], F16, name="xc")

                def blk(t, k):
                    return t[:].rearrange("p (s j) -> p s j", s=2)[
                        :, :, k * D : (k + 1) * D
                    ]

                # level 1 all on DVE (POOL never blocks DVE)
                nc.vector.tensor_add(blk(xm, 0), blk(xt, 0), blk(xt, 1))
                nc.vector.tensor_sub(blk(xm, 1), blk(xt, 0), blk(xt, 1))
                nc.vector.tensor_add(blk(xm, 2), blk(xt, 2), blk(xt, 3))
                nc.vector.tensor_sub(blk(xm, 3), blk(xt, 2), blk(xt, 3))
                # level 2 also on DVE — GpSimd shares an SBUF port with DVE
                # (exclusive lock), so any GpSimd op doubles concurrent DVE ops
                nc.vector.tensor_add(blk(xc, 0), blk(xm, 0), blk(xm, 2))
                nc.vector.tensor_add(blk(xc, 1), blk(xm, 1), blk(xm, 3))
                nc.vector.tensor_sub(blk(xc, 2), blk(xm, 0), blk(xm, 2))
                nc.vector.tensor_sub(blk(xc, 3), blk(xm, 1), blk(xm, 3))
                xcs[b0] = (xc, 0)
                xcs[b0 + 1] = (xc, W)

            def emit_stage_a(s):
                # kron path: t_l[c, dt*128+q] = sum_p xc_l[p, dt*128+c] H128[p,q]
                # tt col layout: l*512 + dt*128 + q
                xc, xo = xcs.pop(s)
                tps = [
                    psum_pool.tile([128, 2 * N], F32, name=f"tps{h}", tag="ps")
                    for h in range(2)
                ]
                for l in range(4):
                    for dt_ in range(4):
                        nc.tensor.matmul(
                            tps[l // 2][
                                :, (l % 2) * N + dt_ * 128 : (l % 2) * N + dt_ * 128 + 128
                            ],
                            xc[:, xo + l * D + dt_ * 128 : xo + l * D + dt_ * 128 + 128],
                            h128_sb[:],
                            start=True,
                            stop=True,
                        )
                tt = t_pool.tile([128, 4 * N], F16, name="tt")
                nc.scalar.copy(tt[:, 0 : 2 * N], tps[0][:])
                nc.scalar.copy(tt[:, 2 * N : 4 * N], tps[1][:])
                tts[s] = ("lmaj", tt)

            def emit_stage_a_full(s):
                # full-H512 left pass: tT[d, n'] with col layout dt*512+l*128+q
                xt, xo = xts.pop(s)
                tps = [
                    psum_pool.tile([128, 2 * N], F32, name=f"tps{h}", tag="ps")
                    for h in range(2)
                ]
                for dt_ in range(4):
                    out = tps[dt_ // 2][:, (dt_ % 2) * N : (dt_ % 2 + 1) * N]
                    for k in range(4):
                        nc.tensor.matmul(
                            out,
                            xt[:, xo + k * D + dt_ * 128 : xo + k * D + dt_ * 128 + 128],
                            h1_sb[:, k * N : (k + 1) * N],
                            start=(k == 0),
                            stop=(k == 3),
                        )
                tt = t_pool.tile([128, 4 * N], F16, name="tt")
                nc.scalar.copy(tt[:, 0 : 2 * N], tps[0][:])
                nc.scalar.copy(tt[:, 2 * N : 4 * N], tps[1][:])
                tts[s] = ("dmaj", tt)

            def emit_pass2(s):
                # Right transform via H512 = H2 (x) H256:
                # U_h[p', k2*256+s] = sum_{dt in {2h,2h+1}} sum_c
                #     tT[dt*128+c, 4p'+k2] H256[(dt&1)*128+c, s]/512
                # y[4p'+k2, ehi*256+s] = U_0 + (-1)^ehi U_1
                layout, tt = tts.pop(s)
                yt = yts.pop(s)
                yo = (s % 2) * W
                pps = [
                    psum_pool.tile([128, 2 * D], F32, name=f"pps{h}", tag="ps")
                    for h in range(2)
                ]
                for k2 in range(4):
                    for h in range(2):
                        for dtl in range(2):
                            dt_ = 2 * h + dtl
                            if layout == "lmaj":
                                lhsT = tt[
                                    :, k2 * N + dt_ * 128 : k2 * N + dt_ * 128 + 128
                                ]
                            else:
                                lhsT = tt[
                                    :, dt_ * N + k2 * 128 : dt_ * N + k2 * 128 + 128
                                ]
                            nc.tensor.matmul(
                                pps[h][:, k2 * 256 : (k2 + 1) * 256],
                                lhsT,
                                h256_sb[:, dtl * 256 : (dtl + 1) * 256],
                                start=(dtl == 0),
                                stop=(dtl == 1),
                            )
                u0 = u_pool.tile([128, 2 * D], F16, name="u0")
                u1 = u_pool.tile([128, 2 * D], F16, name="u1")
                nc.scalar.copy(u0[:], pps[0][:])
                if s % 2 == 0:
                    nc.scalar.copy(u1[:], pps[1][:])
                else:
                    nc.vector.tensor_copy(u1[:], pps[1][:])
                ys = yt[:, yo : yo + W].rearrange("p (k e) -> p k e", e=512)
                u0v = u0[:].rearrange("p (k v) -> p k v", v=256)
                u1v = u1[:].rearrange("p (k v) -> p k v", v=256)
                if s >= BPC - 2:
                    # last pair: combine and store in halves so the final
                    # DMAs start as early as possible
                    for hf in range(2):
                        sl = slice(hf * 2, hf * 2 + 2)
                        nc.vector.tensor_add(ys[:, sl, 0:256], u0v[:, sl], u1v[:, sl])
                        nc.vector.tensor_sub(ys[:, sl, 256:512], u0v[:, sl], u1v[:, sl])
                        nc.sync.dma_start(
                            y_d[s][:, hf * 2 * D : (hf * 2 + 2) * D],
                            yt[:, yo + hf * 2 * D : yo + (hf * 2 + 2) * D],
                        )
                    return
                nc.vector.tensor_add(ys[:, :, 0:256], u0v, u1v)
                nc.vector.tensor_sub(ys[:, :, 256:512], u0v, u1v)
                if s % 2 == 1:
                    b0 = s - 1
                    nc.sync.dma_start(
                        y_d[b0 : b0 + 2].transpose([1, 0, 2]),
                        yt[:].rearrange("p (s j) -> p s j", s=2),
                    )

            # x pairs 0-1 and the H constants are prefetched first; slices
            # 0-3 take the full-H512 left pass (no butterfly dependency) so
            # the PE starts early while the butterfly pipeline fills
            emit_load(0, butterfly=False)
            h1_sb = const_pool.tile([128, 4 * N], F16, tag="h1")
            nc.sync.dma_start(h1_sb[:], h1_d[:])
            emit_load(2)  # pair 1 early so its butterflies fill the pipeline
            h256_sb = const_pool.tile([128, 512], F16, tag="h256")
            nc.sync.dma_start(h256_sb[:], h256_d[:])
            h128_sb = const_pool.tile([128, 128], F16, tag="h128")
            nc.sync.dma_start(h128_sb[:], h128_d[:])

            for s in range(BPC):
                if s % 2 == 0 and s > 2:
                    emit_load(s)
                if s < 2:
                    emit_stage_a_full(s)
                else:
                    emit_stage_a(s)
                if len(pend) >= 2 or (pend and s == BPC - 1):
                    emit_pass2(pend.pop(0))
                    if s == BPC - 1 and pend:
                        emit_pass2(pend.pop(0))
                pend.append(s)
            while pend:
                emit_pass2(pend.pop(0))

    nc.compile()
    return nc


_NC = None


def kernel(x: np.ndarray) -> np.ndarray:
    global _NC, LAST_RESULTS
    if _NC is None:
        _NC = _build()
    x = (
        np.ascontiguousarray(np.asarray(x), dtype=np.float32)
        .astype(np.float16)
        .reshape(NCORES, BPC, 128, 4 * D)
    )
    H = _hadamard(N)
    h128 = np.ascontiguousarray(_hadamard(128)).astype(np.float16)
    h1 = np.ascontiguousarray(
        H.reshape(128, 4, 128, 4).transpose(0, 1, 3, 2).reshape(128, 4 * N)
    ).astype(np.float16)
    H256 = _hadamard(256)
    h256 = np.ascontiguousarray(
        H256.reshape(2, 128, 256).transpose(1, 0, 2).reshape(128, 512)
        / np.float32(512.0)
    ).astype(np.float16)
    in_maps = [{"x": x[i], "h128": h128, "h1": h1, "h256": h256} for i in range(NCORES)]
    trace = os.environ.get("KERNEL_TRACE", "") == "1"
    res = run_bass_kernel_spmd(_NC, in_maps, list(range(NCORES)), trace=trace)
    LAST_RESULTS = res
    out = np.stack([np.asarray(r["y"]) for r in res.results], axis=0)
    return out.reshape(B, N, D).astype(np.float32)
